# revision 9
# baseline (speedup 1.0000x reference)
"""Trainium2 Bass kernel for nn_Attention_60344290509655 (additive/Bahdanau-style
attention with [B,T,T,H] tanh intermediate, causal mask, custom softmax).

Strategy (8 NeuronCores, data-parallel):
  core = 2*b + parity handles batch b and 256 queries: 4 groups of 64
  (query units u_g = 2g + parity, g=0..3). Causal span of group g is
  exactly g+1 s-chunks of 128 keys for BOTH parities -> all 8 cores run
  one identical graph; per-core differences live in the input data only.

Math notes:
  - rowmax subtraction in the reference cancels exactly (scores bounded by
    sum|Wa| ~ 6, exp can't overflow) except through the +1e-7 epsilon,
    where the relative effect is ~1e-5. We skip it.
  - tanh is evaluated only on the (block-)causal region: ~half the work.
  - bf16 staging for the tanh pipeline; f32 accumulation in PSUM.

Per-core pipeline per (g, c) chunk:
  DVE  : wide[128,4096] = K2[:,chunk] (+bcast) + QB[:,group pairs]  (bf16)
  ACT  : tanh(wide)
  PE   : 32 matmuls lhsT=tanh[:,128j:..] rhs=Wa2[128,2] -> E_T[128s, 64q] f32
  ACT  : exp -> bf16
  DVE  : causal mask multiply (final chunk only)
  PE   : [V|1] fused matmul accumulate -> [64q, 129] (attn*V plus row sums)
  PE   : transpose -> e output rows
"""
import sys
import numpy as np
import ml_dtypes

for _p in ("/opt/trn_rl_repo", "/root/.axon_site/_ro/trn_rl_repo"):
    if _p not in sys.path:
        sys.path.insert(0, _p)

import concourse.bass as bass
import concourse.bacc as bacc
import concourse.mybir as mybir
from concourse import tile
from concourse.bass_utils import run_bass_kernel_spmd

BF = mybir.dt.bfloat16
F32 = mybir.dt.float32
B, T, D, H = 4, 512, 128, 64
NCORE = 8
NG = 4          # query groups per core
GQ = 64         # queries per group
NPAIR = GQ // 2  # 32 pairs per group
SC = 128        # s-chunk width

# packed f32 param blob columns: Wt | Wx | Wd | bh | demo | identity
PF_WT, PF_WX, PF_WD, PF_BH, PF_DM, PF_ID = 0, 64, 128, 192, 193, 194
PF_N = 194 + 128


# packed bf16 blob columns: wa2 | mask | identity
PB_WA, PB_MK, PB_ID = 0, 2, 66
PB_N = 66 + 128


def build_nc():
    nc = bacc.Bacc()
    xb_e = nc.declare_dram_parameter("xb", [T, D], F32, isOutput=False)
    xq_e = nc.declare_dram_parameter("xq", [NG * GQ, D], F32, isOutput=False)
    pf_e = nc.declare_dram_parameter("pf", [128, PF_N], F32, isOutput=False)
    pb_e = nc.declare_dram_parameter("pb", [128, PB_N], BF, isOutput=False)
    v_e = nc.declare_dram_parameter("v_out", [NG * GQ, D], F32, isOutput=True)
    e_e = nc.declare_dram_parameter("e_out", [NG * GQ, T], F32, isOutput=True)

    with tile.TileContext(nc) as tc:
        with tc.tile_pool(name="const", bufs=1) as const:
            k2 = const.tile([128, T], BF, tag="k2")
            qb = const.tile([128, NG * NPAIR], BF, tag="qb")
            pb = const.tile([128, PB_N], BF, tag="pb")
            vb = [const.tile([128, D + 1], BF, tag=f"vb{c}", name=f"vb{c}")
                  for c in range(4)]
            nc.sync.dma_start(pb[:], pb_e[:])
            wa2 = pb[:, PB_WA:PB_WA + 2]
            maskt = pb[:, PB_MK:PB_MK + GQ]
            idb = pb[:, PB_ID:PB_ID + 128]

            # ---------- setup: projections, transposes, packing ----------
            with (
                tc.tile_pool(name="stmp", bufs=1) as stmp,
                tc.tile_pool(name="sps", bufs=1, space=bass.MemorySpace.PSUM) as sps,
            ):
                pf = stmp.tile([128, PF_N], F32, tag="pf")
                nc.sync.dma_start(pf[:], pf_e[:])
                idf = pf[:, PF_ID:PF_ID + 128]

                xt = [stmp.tile([128, D], F32, tag=f"xt{i}", name=f"xt{i}")
                      for i in range(4)]
                for i in range(4):
                    nc.sync.dma_start(xt[i][:], xb_e[128 * i:128 * (i + 1), :])
                xqt = [stmp.tile([128, D], F32, tag=f"xqt{i}", name=f"xqt{i}")
                       for i in range(2)]
                for i in range(2):
                    nc.sync.dma_start(xqt[i][:], xq_e[128 * i:128 * (i + 1), :])

                # V tiles [128, 129] bf16 with trailing ones column
                for c in range(4):
                    nc.vector.tensor_copy(vb[c][:, 0:D], xt[c][:])
                    nc.gpsimd.memset(vb[c][:, D:D + 1], 1.0)

                # X^T for projections
                xbT = stmp.tile([128, T], F32, tag="xbT")
                for c in range(4):
                    tp = sps.tile([128, 128], F32, tag="tp")
                    nc.tensor.transpose(tp[:], xt[c][:], idf)
                    nc.vector.tensor_copy(xbT[:, 128 * c:128 * (c + 1)], tp[:])
                xqT = stmp.tile([128, NG * GQ], F32, tag="xqT")
                for c in range(2):
                    tp = sps.tile([128, 128], F32, tag="tp")
                    nc.tensor.transpose(tp[:], xqt[c][:], idf)
                    nc.vector.tensor_copy(xqT[:, 128 * c:128 * (c + 1)], tp[:])

                # K^T = Wx^T @ X^T -> [H, T]; duplicate rows into k2
                ktp = sps.tile([H, T], F32, tag="ktp")
                nc.tensor.matmul(ktp[:], pf[:, PF_WX:PF_WX + H], xbT[:],
                                 start=True, stop=True)
                nc.vector.tensor_copy(k2[0:H, :], ktp[:])
                nc.vector.tensor_copy(k2[H:2 * H, :], ktp[:])

                # db = Wd^T @ demo + bh
                dbp = sps.tile([H, 1], F32, tag="dbp")
                nc.tensor.matmul(dbp[:], pf[0:12, PF_WD:PF_WD + H],
                                 pf[0:12, PF_DM:PF_DM + 1], start=True, stop=True)
                db_sb = stmp.tile([H, 1], F32, tag="db_sb")
                nc.scalar.activation(db_sb[:], dbp[:],
                                     mybir.ActivationFunctionType.Identity,
                                     bias=pf[0:H, PF_BH:PF_BH + 1])

                # Q^T = Wt^T @ Xq^T + db -> pack pairs into qb
                qtp = sps.tile([H, NG * GQ], F32, tag="qtp")
                nc.tensor.matmul(qtp[:], pf[:, PF_WT:PF_WT + H], xqT[:],
                                 start=True, stop=True)
                qtb = stmp.tile([H, NG * GQ], F32, tag="qtb")
                nc.vector.tensor_scalar(qtb[:], qtp[:], db_sb[:], None,
                                        op0=mybir.AluOpType.add)
                nc.vector.tensor_copy(qb[0:H, :], qtb[:, 0:NG * GQ:2])
                nc.vector.tensor_copy(qb[H:2 * H, :], qtb[:, 1:NG * GQ:2])

            # ---------- main: 10 uniform chunks ----------
            with (
                tc.tile_pool(name="widep", bufs=3) as widep,
                tc.tile_pool(name="tanhp", bufs=3) as tanhp,
                tc.tile_pool(name="esbp", bufs=3) as esbp,
                tc.tile_pool(name="emp", bufs=2) as emp,
                tc.tile_pool(name="etp", bufs=3, space=bass.MemorySpace.PSUM) as etp,
                tc.tile_pool(name="vsp", bufs=2, space=bass.MemorySpace.PSUM) as vsp,
                tc.tile_pool(name="eqp", bufs=2, space=bass.MemorySpace.PSUM) as eqp,
                tc.tile_pool(name="outp", bufs=2) as outp,
            ):
                for g in range(NG):
                    vs_ps = vsp.tile([GQ, D + 1], F32, tag="vs")
                    eq_ps = eqp.tile([GQ, T], BF, tag="eq")
                    for c in range(g + 1):
                        wide = widep.tile([128, NPAIR * SC], BF, tag="wide")
                        k2_rep = (k2[:, SC * c:SC * (c + 1)]
                                  .rearrange("p (j w) -> p j w", j=1)
                                  .to_broadcast((128, NPAIR, SC)))
                        qb_rep = (qb[:, NPAIR * g:NPAIR * (g + 1)]
                                  .rearrange("p (j w) -> p j w", w=1)
                                  .to_broadcast((128, NPAIR, SC)))
                        nc.vector.tensor_tensor(
                            wide[:, :].rearrange("p (j w) -> p j w", j=NPAIR),
                            k2_rep, qb_rep, mybir.AluOpType.add)
                        tw = tanhp.tile([128, NPAIR * SC], BF, tag="tw")
                        nc.scalar.activation(tw[:], wide[:],
                                             mybir.ActivationFunctionType.Tanh)
                        et_ps = etp.tile([SC, GQ], F32, tag="et")
                        for j in range(NPAIR):
                            nc.tensor.matmul(et_ps[:, 2 * j:2 * j + 2],
                                             tw[:, SC * j:SC * (j + 1)], wa2,
                                             start=True, stop=True)
                        e_sb = esbp.tile([SC, GQ], BF, tag="esb")
                        nc.scalar.activation(e_sb[:], et_ps[:],
                                             mybir.ActivationFunctionType.Exp)
                        if c == g:
                            em = emp.tile([SC, GQ], BF, tag="em")
                            nc.vector.tensor_tensor(em[:], e_sb[:], maskt,
                                                    mybir.AluOpType.mult)
                        else:
                            em = e_sb
                        nc.tensor.matmul(vs_ps[:], em[:], vb[c][:],
                                         start=(c == 0), stop=(c == g),
                                         skip_group_check=True)
                        nc.tensor.transpose(eq_ps[:, SC * c:SC * (c + 1)],
                                            em[:], idb)
                    # group epilogue: normalize, write v and e rows
                    s_sb = outp.tile([GQ, 1], F32, tag="s")
                    nc.vector.tensor_scalar(s_sb[:], vs_ps[:, D:D + 1], 1e-7, None,
                                            op0=mybir.AluOpType.add)
                    rinv = outp.tile([GQ, 1], F32, tag="rinv")
                    nc.vector.reciprocal(rinv[:], s_sb[:])
                    v_sb = outp.tile([GQ, D], F32, tag="v")
                    nc.vector.tensor_scalar(v_sb[:], vs_ps[:, 0:D], rinv[:], None,
                                            op0=mybir.AluOpType.mult)
                    nc.sync.dma_start(v_e[GQ * g:GQ * (g + 1), :], v_sb[:])
                    e_sb_out = outp.tile([GQ, T], F32, tag="eout")
                    W = SC * (g + 1)
                    if W < T:
                        nc.gpsimd.memset(e_sb_out[:, W:T], 0.0)
                    nc.vector.tensor_scalar(e_sb_out[:, 0:W], eq_ps[:, 0:W], rinv[:],
                                            None, op0=mybir.AluOpType.mult)
                    nc.sync.dma_start(e_e[GQ * g:GQ * (g + 1), :], e_sb_out[:])
    nc.finalize()
    return nc


def _bf(x):
    return np.asarray(x, dtype=ml_dtypes.bfloat16)


def host_inputs(input, demo, Wt, Wx, Wd, bh, Wa):
    """Build the 8 per-core input maps."""
    input = np.asarray(input, np.float32)
    demo = np.asarray(demo, np.float32)
    wa = np.asarray(Wa, np.float32)[:, 0]

    pb = np.zeros((2, 128, PB_N), np.float32)   # per parity
    pb[:, 0:H, PB_WA + 0] = wa
    pb[:, H:2 * H, PB_WA + 1] = wa
    sp = np.arange(128)[:, None]
    qf = np.arange(GQ)[None, :]
    for par in range(2):
        pb[par, :, PB_MK:PB_MK + GQ] = (sp <= 64 * par + qf)
    pb[:, :, PB_ID:PB_ID + 128] = np.eye(128, dtype=np.float32)

    pf_base = np.zeros((128, PF_N), np.float32)
    pf_base[:, PF_WT:PF_WT + H] = np.asarray(Wt, np.float32)
    pf_base[:, PF_WX:PF_WX + H] = np.asarray(Wx, np.float32)
    pf_base[0:12, PF_WD:PF_WD + H] = np.asarray(Wd, np.float32)
    pf_base[0:H, PF_BH] = np.asarray(bh, np.float32)
    pf_base[:, PF_ID:PF_ID + 128] = np.eye(128, dtype=np.float32)

    in_maps = []
    qrows_all = []
    for core in range(NCORE):
        b, par = core // 2, core % 2
        units = [2 * g + par for g in range(NG)]
        qrows = np.concatenate([np.arange(64 * u, 64 * u + 64) for u in units])
        qrows_all.append((b, qrows))
        pf = pf_base.copy()
        pf[0:12, PF_DM] = demo[b]
        in_maps.append({
            "xb": input[b],
            "xq": np.ascontiguousarray(input[b][qrows]),
            "pf": pf,
            "pb": _bf(pb[par]),
        })
    return in_maps, qrows_all


_NC_CACHE = []


def kernel(input, demo, Wt, Wx, Wd, bh, Wa, ba):
    if not _NC_CACHE:
        _NC_CACHE.append(build_nc())
    nc = _NC_CACHE[0]
    in_maps, qrows_all = host_inputs(input, demo, Wt, Wx, Wd, bh, Wa)
    res = run_bass_kernel_spmd(nc, in_maps, core_ids=list(range(NCORE)))
    v = np.zeros((B, T, D), np.float32)
    e = np.zeros((B, T, T), np.float32)
    for core in range(NCORE):
        b, qrows = qrows_all[core]
        v[b][qrows] = res.results[core]["v_out"]
        e[b][qrows] = res.results[core]["e_out"]
    return (v, e)
